# revision 3
# baseline (speedup 1.0000x reference)
"""Binarized 3x3 conv2d (hardtanh activation, clipped reweight ++ plain conv_w)
run data-parallel across 8 Trainium2 NeuronCores.

Math: out = conv2d(clip(x,-1,1), concat(clip(reweight,-1,1), conv_w)), pad=1
— a single 128->128 channel 3x3 conv (the two branches just split the output
channels), so the weights are fused + pre-transposed on the host and the conv
runs as one kernel.

Sharding: pure data parallel — batch 32 -> 4 images per core, weights
replicated (tiny). No collectives needed (forward only).

Mode "fp8dr" (default): self-correcting fp8 DoubleRow conv.
  The PE's only >1x matmul mode on TRN2 is fp8e4/e5 DoubleRow (2 MACs/cell/
  cycle, K virtually 256).  Plain e4m3 quantization of weights+activations
  fails the 2e-2 absmax-rel budget (measured 3.0e-2, dominated by the weight
  quantization).  Instead each DoubleRow matmul computes BOTH the fp8 product
  and its first-order weight-error correction in one pass:
      slot0: Whi^T  @ a8        with Whi = e4m3(W)
      slot1: Wlo^T  @ (a8/16)   with Wlo = e4m3(16*(W - Whi))
  = (Whi + (W-Whi))^T a8 exactly (to second order), so only the activation
  quantization error remains (measured 1.4e-2 < 2e-2).  Activations live as a
  two-plane fp8 image [C, 2, 114, 114]: plane0 = e4m3(clip(x)) (DVE clip),
  plane1 = plane0/16 (ACT scale-copy).  Each of the 9 taps is one DoubleRow
  matmul whose moving operand is the 4D AP [C, 2(plane), rows, 112].

  LDWEIGHTS in DoubleRow mode can't use Fast Weight Load (256 cols, ~256 cyc)
  and would eat the 2x gain, so blocks are processed in groups of 8 (one PSUM
  bank each) with the tap loop outer: the first matmul of a tap self-loads,
  the other 7 set InstMatmult.ldweights=False and reuse the loaded weights
  (measured: 148.6us fp16 -> 103.8us for the grouped DR inner loop).

  I/O is fp16 (pure dtype cast on host, float16 in / float16 out) to keep DMA
  at ~26MB/core, under the PE time.  PSUM (fp32) is drained by ACT copies to
  fp16 SBUF tiles and DMA'd out from there (same engine => program-order WAR,
  keeps every instruction within the 2-sync-command hardware limit).

Mode "fp16": previous baseline (9 normal fp16 matmuls per block, fp32 I/O),
  kept as fallback.  Measured 163.6us this session.
"""

import time as _time

import numpy as np
import ml_dtypes
from contextlib import ExitStack

import concourse.bass as bass
import concourse.mybir as mybir
import concourse.tile as tile
from concourse import bacc

B, C, H, W = 32, 128, 112, 112
NCORES = 8
BPC = B // NCORES  # images per core
R = 4              # output rows per PSUM block (R*W = 448 <= 512 psum bank)
G = 8              # blocks per ldweights-amortization group (= PSUM banks)

MODE = "fp8dr"  # "fp8dr" | "fp16"

F8 = ml_dtypes.float8_e4m3  # TRN FP8_EXP4-compatible for |v| <= 240

_nc_cache: dict = {}
_runner_cache: dict = {}

# (0,0) first: it is never row-trimmed, so the start=True matmul writes
# every element of the PSUM block before partial taps accumulate into it
TAPS = [(0, 0)] + [
    (kh, kw) for kh in (-1, 0, 1) for kw in (-1, 0, 1) if (kh, kw) != (0, 0)
]


def _build_fp8dr(bpc: int = BPC, h: int = H, w: int = W, reps: int = 1) -> bass.Bass:
    f32 = mybir.dt.float32
    f16 = mybir.dt.float16
    f8 = mybir.dt.float8e4
    DR = mybir.MatmulPerfMode.DoubleRow
    hp, wp = h + 2, w + 2
    assert h % R == 0
    nblocks = h // R

    nc = bacc.Bacc("TRN2", target_bir_lowering=False, debug=False)
    x_in = nc.declare_dram_parameter("x", [bpc, C, h, w], f16, isOutput=False)
    w_in = nc.declare_dram_parameter("w", [C, 9, 2, C], f8, isOutput=False)
    out_d = nc.declare_dram_parameter("out", [bpc, C, h, w], f16, isOutput=True)

    with tile.TileContext(nc) as tc, ExitStack() as ctx:
        wpool = ctx.enter_context(tc.tile_pool(name="wpool", bufs=1))
        apool = ctx.enter_context(tc.tile_pool(name="apool", bufs=2))
        opool = ctx.enter_context(tc.tile_pool(name="opool", bufs=6))
        pspool = ctx.enter_context(tc.tile_pool(name="pspool", bufs=1, space="PSUM"))

        w_s = wpool.tile([C, 9, 2, C], f8)
        nc.sync.dma_start(w_s[:], w_in[:])

        # PE warmup: the HAM clock gate holds the PE at low clock until it has
        # been busy ~3.4 us.  A few dummy matmuls during the input-DMA lead-in
        # (PE would be idle anyway) un-throttle it before the real work lands.
        warm = wpool.tile([C, R * w], f8)
        nc.vector.memset(warm[:], 0.0)
        wps = pspool.tile([C, R * w], f32, tag="ps0", name="wps")
        for _wi in range(6):
            nc.tensor.matmul(wps[:], warm[:, :C], warm[:], start=True, stop=True)

        for b_outer in range(bpc * reps):
            b = b_outer % bpc
            a_raw = apool.tile([C, h, w], f16, tag="a_raw", bufs=2)
            a = apool.tile([C, 2, hp, wp], f8, tag="a_pad", bufs=4)
            # zero the 1-wide border of both planes
            nc.vector.memset(a[:, :, 0, :], 0.0)
            nc.vector.memset(a[:, :, hp - 1, :], 0.0)
            nc.vector.memset(a[:, :, 1 : hp - 1, 0], 0.0)
            nc.vector.memset(a[:, :, 1 : hp - 1, wp - 1], 0.0)
            # image 0 leads with smaller chunks so the first matmul group can
            # start earlier; later images are fully overlapped with compute
            if h >= 32 and b_outer == 0:
                sched = [2, 2, 4, 8] + [16] * ((h - 16) // 16)
            elif h >= 32:
                sched = [16] * (h // 16)
            else:
                sched = [h]
            bounds = [0]
            for sz in sched:
                bounds.append(bounds[-1] + sz)
            for ci in range(len(sched)):
                r0, r1 = bounds[ci], bounds[ci + 1]
                nc.sync.dma_start(a_raw[:, r0:r1, :], x_in[b][:, r0:r1, :])
                # plane0 = e4m3(clip(x)) on DVE
                nc.vector.tensor_scalar(
                    out=a[:, 0, 1 + r0 : 1 + r1, 1 : wp - 1], in0=a_raw[:, r0:r1, :],
                    scalar1=1.0, scalar2=-1.0,
                    op0=mybir.AluOpType.min, op1=mybir.AluOpType.max,
                )
                # plane1 = plane0 / 16 on ACT
                nc.scalar.activation(
                    out=a[:, 1, 1 + r0 : 1 + r1, 1 : wp - 1],
                    in_=a[:, 0, 1 + r0 : 1 + r1, 1 : wp - 1],
                    func=mybir.ActivationFunctionType.Copy, scale=0.0625,
                )

            out_flat = out_d[b].rearrange("c h w -> c (h w)")
            # group schedule: image 0 ramps up with small groups
            if b_outer == 0:
                gsched = [1, 1, 2, 4] + [G] * ((nblocks - 8) // G)
            else:
                gsched = [G] * (nblocks // G)
            while sum(gsched) < nblocks:
                gsched.append(nblocks - sum(gsched))
            g0 = 0
            for gsz in gsched:
                blks = list(range(g0, g0 + gsz))
                g0 += gsz
                pss = []
                for j, blk in enumerate(blks):
                    ps_t = pspool.tile([C, R * w], f32, tag=f"ps{j}", name=f"ps{j}_")
                    pss.append(ps_t)
                for i, (kh, kw) in enumerate(TAPS):
                    lhsT = w_s[:, (kh + 1) * 3 + (kw + 1), :, :]
                    for j, blk in enumerate(blks):
                        h0 = blk * R
                        # trim rows that would only read the zero pad (image
                        # top/bottom edges); the trimmed PSUM slice stays flat
                        # because only whole rows are dropped
                        rr0 = max(0, -(h0 + kh))
                        rr1 = min(R, h - h0 - kh)
                        rhs = a[:, :, h0 + kh + 1 + rr0 : h0 + kh + 1 + rr1,
                                kw + 1 : kw + 1 + w]
                        inst = nc.tensor.matmul(
                            pss[j][:, rr0 * w : rr1 * w], lhsT, rhs,
                            start=(i == 0), stop=(i == len(TAPS) - 1),
                            perf_mode=DR,
                        )
                        if j > 0:
                            inst.ldweights = False
                for j, blk in enumerate(blks):
                    h0 = blk * R
                    ot = opool.tile([C, R * w], f16, tag="ot")
                    nc.scalar.copy(ot[:], pss[j][:])
                    nc.scalar.dma_start(out_flat[:, h0 * w : (h0 + R) * w], ot[:])

    nc.compile()
    return nc


def _build_fp16(bpc: int = BPC, h: int = H, w: int = W, reps: int = 1) -> bass.Bass:
    f32 = mybir.dt.float32
    wdt = mybir.dt.float16
    hp, wp = h + 2, w + 2
    assert h % R == 0

    nc = bacc.Bacc("TRN2", target_bir_lowering=False, debug=False)
    x_in = nc.declare_dram_parameter("x", [bpc, C, h, w], f32, isOutput=False)
    w_in = nc.declare_dram_parameter("w", [C, 9, C], wdt, isOutput=False)
    out_d = nc.declare_dram_parameter("out", [bpc, C, h, w], f32, isOutput=True)

    with tile.TileContext(nc) as tc, ExitStack() as ctx:
        wpool = ctx.enter_context(tc.tile_pool(name="wpool", bufs=1))
        apool = ctx.enter_context(tc.tile_pool(name="apool", bufs=2))
        opool = ctx.enter_context(tc.tile_pool(name="opool", bufs=6))
        pspool = ctx.enter_context(tc.tile_pool(name="pspool", bufs=8, space="PSUM"))

        w_s = wpool.tile([C, 9, C], wdt)
        nc.sync.dma_start(w_s[:], w_in[:])

        warm = wpool.tile([C, R * w], wdt)
        nc.vector.memset(warm[:], 0.0)
        wps = pspool.tile([C, R * w], f32, tag="ps")
        for _wi in range(6):
            nc.tensor.matmul(wps[:], warm[:, :C], warm[:], start=True, stop=True)

        for b_outer in range(bpc * reps):
            b = b_outer % bpc
            a_raw = apool.tile([C, h, w], f32, tag="a_raw", bufs=1)
            a = apool.tile([C, hp, wp], wdt, tag="a_pad", bufs=4)
            nc.vector.memset(a[:, 0, :], 0.0)
            nc.vector.memset(a[:, hp - 1, :], 0.0)
            nc.vector.memset(a[:, 1 : hp - 1, 0], 0.0)
            nc.vector.memset(a[:, 1 : hp - 1, wp - 1], 0.0)
            if h >= 32 and b_outer == 0:
                sched = [2, 2, 4, 8] + [16] * ((h - 16) // 16)
            elif h >= 32:
                sched = [16] * (h // 16)
            else:
                sched = [h]
            bounds = [0]
            for sz in sched:
                bounds.append(bounds[-1] + sz)
            for ci in range(len(sched)):
                r0, r1 = bounds[ci], bounds[ci + 1]
                nc.sync.dma_start(a_raw[:, r0:r1, :], x_in[b][:, r0:r1, :])
                nc.vector.tensor_scalar(
                    out=a[:, 1 + r0 : 1 + r1, 1 : wp - 1], in0=a_raw[:, r0:r1, :],
                    scalar1=1.0, scalar2=-1.0,
                    op0=mybir.AluOpType.min, op1=mybir.AluOpType.max,
                )

            out_flat = out_d[b].rearrange("c h w -> c (h w)")
            for h0 in range(0, h, R):
                ps = pspool.tile([C, R * w], f32)
                for i, (kh, kw) in enumerate(TAPS):
                    rr0 = max(0, -(h0 + kh))
                    rr1 = min(R, h - h0 - kh)
                    rhs = a[:, h0 + kh + 1 + rr0 : h0 + kh + 1 + rr1,
                            kw + 1 : kw + 1 + w]
                    lhsT = w_s[:, (kh + 1) * 3 + (kw + 1), :]
                    nc.tensor.matmul(
                        ps[:, rr0 * w : rr1 * w], lhsT, rhs,
                        start=(i == 0), stop=(i == len(TAPS) - 1),
                    )

                ot = opool.tile([C, R * w], f32)
                nc.scalar.copy(ot[:], ps[:])
                nc.scalar.dma_start(out_flat[:, h0 * w : (h0 + R) * w], ot[:])

    nc.compile()
    return nc


def _build(mode: str, bpc: int = BPC, h: int = H, w: int = W, reps: int = 1) -> bass.Bass:
    if mode == "fp8dr":
        return _build_fp8dr(bpc=bpc, h=h, w=w, reps=reps)
    return _build_fp16(bpc=bpc, h=h, w=w, reps=reps)


def _prep_weights(reweight: np.ndarray, conv_w: np.ndarray, mode: str) -> np.ndarray:
    w_full = np.concatenate([np.clip(reweight, -1.0, 1.0), conv_w], axis=0)  # [128,128,3,3]
    w_t = np.ascontiguousarray(w_full.transpose(1, 2, 3, 0)).reshape(C, 9, C)  # [ci,tap,co]
    if mode == "fp8dr":
        w_t = w_t.astype(np.float32)
        whi = w_t.astype(F8)
        wlo = ((w_t - whi.astype(np.float32)) * 16.0).astype(F8)
        return np.ascontiguousarray(np.stack([whi, wlo], axis=2))  # [ci,tap,2,co]
    return w_t.astype(np.float16)


def _make_in_maps(x: np.ndarray, reweight: np.ndarray, conv_w: np.ndarray, mode: str):
    w_k = _prep_weights(np.asarray(reweight), np.asarray(conv_w), mode)
    xdt = np.float16 if mode == "fp8dr" else np.float32
    x = np.asarray(x).astype(xdt)
    return [
        {"x": np.ascontiguousarray(x[i * BPC : (i + 1) * BPC]), "w": w_k}
        for i in range(NCORES)
    ]


def _get_nc(mode: str):
    if mode not in _nc_cache:
        _nc_cache[mode] = _build(mode)
    return _nc_cache[mode]


class _Runner:
    """Persistent jitted shard_map executor for one compiled Bass module.

    Mirrors concourse.bass2jax.run_bass_via_pjrt's multi-core path, but keeps
    the jitted function (and the donated output buffers) alive across calls so
    repeated kernel() invocations skip recompilation.  Output buffers are
    donation-chained: the kernel writes every output element, so reusing the
    previous call's outputs as the next call's output buffers is safe.
    """

    def __init__(self, nc, n_cores: int):
        import jax
        from concourse import bass2jax
        from jax.experimental.shard_map import shard_map
        from jax.sharding import Mesh, NamedSharding, PartitionSpec

        bass2jax.install_neuronx_cc_hook()
        self.jax = jax
        self.n_cores = n_cores
        partition_name = nc.partition_id_tensor.name if nc.partition_id_tensor else None
        in_names, out_names, out_avals = [], [], []
        for alloc in nc.m.functions[0].allocations:
            if not isinstance(alloc, mybir.MemoryLocationSet):
                continue
            name = alloc.memorylocations[0].name
            if alloc.kind == "ExternalInput":
                if name != partition_name:
                    in_names.append(name)
            elif alloc.kind == "ExternalOutput":
                out_names.append(name)
                out_avals.append(
                    jax.core.ShapedArray(
                        tuple(alloc.tensor_shape), mybir.dt.np(alloc.dtype)
                    )
                )
        self.in_names, self.out_names, self.out_avals = in_names, out_names, out_avals
        n_params = len(in_names)
        all_in_names = list(in_names) + list(out_names)
        if partition_name is not None:
            all_in_names.append(partition_name)
        donate = tuple(range(n_params, n_params + len(out_names)))

        def _body(*args):
            operands = list(args)
            if partition_name is not None:
                operands.append(bass2jax.partition_id_tensor())
            return tuple(
                bass2jax._bass_exec_p.bind(
                    *operands,
                    out_avals=tuple(out_avals),
                    in_names=tuple(all_in_names),
                    out_names=tuple(out_names),
                    lowering_input_output_aliases=(),
                    sim_require_finite=True,
                    sim_require_nnan=True,
                    nc=nc,
                )
            )

        devices = jax.devices()[:n_cores]
        assert len(devices) >= n_cores, f"need {n_cores} devices, got {len(devices)}"
        mesh = Mesh(np.asarray(devices), ("core",))
        self.sharding = NamedSharding(mesh, PartitionSpec("core"))
        self.sharded = jax.jit(
            shard_map(
                _body, mesh=mesh,
                in_specs=(PartitionSpec("core"),) * (n_params + len(out_names)),
                out_specs=(PartitionSpec("core"),) * len(out_names),
                check_rep=False,
            ),
            donate_argnums=donate, keep_unused=True,
        )
        self._outs = None  # donated output buffers, chained across calls

    def __call__(self, in_maps):
        jax = self.jax
        per_core = [[np.asarray(m[name]) for name in self.in_names] for m in in_maps]
        concat_in = [
            np.concatenate([per_core[c][i] for c in range(self.n_cores)], axis=0)
            for i in range(len(self.in_names))
        ]
        xin = [jax.device_put(a, self.sharding) for a in concat_in]
        if self._outs is None:
            self._outs = [
                jax.device_put(
                    np.zeros((self.n_cores * av.shape[0], *av.shape[1:]), av.dtype),
                    self.sharding,
                )
                for av in self.out_avals
            ]
        self._outs = list(self.sharded(*xin, *self._outs))
        out_np = [np.asarray(o) for o in self._outs]
        return [
            {
                name: out_np[i].reshape(self.n_cores, *self.out_avals[i].shape)[c]
                for i, name in enumerate(self.out_names)
            }
            for c in range(self.n_cores)
        ]


def _run_spmd(nc, in_maps, mode: str):
    last = None
    for attempt in range(3):
        try:
            if mode not in _runner_cache:
                _runner_cache[mode] = _Runner(nc, NCORES)
            return _runner_cache[mode](in_maps)
        except Exception as e:
            last = e
            _runner_cache.pop(mode, None)
        # fall back to the stock one-shot path (also covers transient
        # device/terminal wedges, with a pause between attempts)
        try:
            from concourse.bass_utils import run_bass_kernel_spmd

            return run_bass_kernel_spmd(nc, in_maps, list(range(NCORES))).results
        except Exception as e:
            last = e
            _time.sleep(15)
    raise last


def run(x, reweight, conv_w, mode: str | None = None):
    mode = mode or MODE
    nc = _get_nc(mode)
    in_maps = _make_in_maps(x, reweight, conv_w, mode)
    results = _run_spmd(nc, in_maps, mode)
    out = np.concatenate([results[i]["out"] for i in range(NCORES)], axis=0)
    if out.dtype != np.float32:
        out = out.astype(np.float32)
    return out


def kernel(x, reweight, conv_w):
    return run(x, reweight, conv_w)


# revision 4
# speedup vs baseline: 1.1217x; 1.1217x over previous
"""Binarized 3x3 conv2d (hardtanh activation, clipped reweight ++ plain conv_w)
run data-parallel across 8 Trainium2 NeuronCores.

Math: out = conv2d(clip(x,-1,1), concat(clip(reweight,-1,1), conv_w)), pad=1
— a single 128->128 channel 3x3 conv (the two branches just split the output
channels), so the weights are fused + pre-transposed on the host and the conv
runs as one kernel.

Sharding: pure data parallel — batch 32 -> 4 images per core, weights
replicated (tiny). No collectives needed (forward only).

Mode "fp8dr" (default): self-correcting fp8 DoubleRow conv.
  The PE's only >1x matmul mode on TRN2 is fp8e4/e5 DoubleRow (2 MACs/cell/
  cycle, K virtually 256).  Plain e4m3 quantization of weights+activations
  fails the 2e-2 absmax-rel budget (measured 3.0e-2, dominated by the weight
  quantization).  Instead each DoubleRow matmul computes BOTH the fp8 product
  and its first-order weight-error correction in one pass:
      slot0: Whi^T  @ a8        with Whi = e4m3(W)
      slot1: Wlo^T  @ (a8/16)   with Wlo = e4m3(16*(W - Whi))
  = (Whi + (W-Whi))^T a8 exactly (to second order), so only the activation
  quantization error remains (measured 1.4e-2 < 2e-2).  Activations live as a
  two-plane fp8 image [C, 2, 114, 114]: plane0 = e4m3(clip(x)) (DVE clip),
  plane1 = plane0/16 (ACT scale-copy).  Each of the 9 taps is one DoubleRow
  matmul whose moving operand is the 4D AP [C, 2(plane), rows, 112].

  LDWEIGHTS in DoubleRow mode can't use Fast Weight Load (256 cols, ~256 cyc)
  and would eat the 2x gain, so blocks are processed in groups of 8 (one PSUM
  bank each) with the tap loop outer: the first matmul of a tap self-loads,
  the other 7 set InstMatmult.ldweights=False and reuse the loaded weights
  (measured: 148.6us fp16 -> 103.8us for the grouped DR inner loop).

  I/O is fp16 (pure dtype cast on host, float16 in / float16 out) to keep DMA
  at ~26MB/core, under the PE time.  PSUM (fp32) is drained by ACT copies to
  fp16 SBUF tiles and DMA'd out from there (same engine => program-order WAR,
  keeps every instruction within the 2-sync-command hardware limit).

Mode "fp16": previous baseline (9 normal fp16 matmuls per block, fp32 I/O),
  kept as fallback.  Measured 163.6us this session.
"""

import time as _time

import numpy as np
import ml_dtypes
from contextlib import ExitStack

import concourse.bass as bass
import concourse.mybir as mybir
import concourse.tile as tile
from concourse import bacc

B, C, H, W = 32, 128, 112, 112
NCORES = 8
BPC = B // NCORES  # images per core
R = 4              # output rows per PSUM block (R*W = 448 <= 512 psum bank)
G = 8              # blocks per ldweights-amortization group (= PSUM banks)

MODE = "fp8dr"  # "fp8dr" | "fp16"

F8 = ml_dtypes.float8_e4m3  # TRN FP8_EXP4-compatible for |v| <= 240

_nc_cache: dict = {}
_runner_cache: dict = {}

# (0,0) first: it is never row-trimmed, so the start=True matmul writes
# every element of the PSUM block before partial taps accumulate into it
TAPS = [(0, 0)] + [
    (kh, kw) for kh in (-1, 0, 1) for kw in (-1, 0, 1) if (kh, kw) != (0, 0)
]


def _build_fp8dr(bpc: int = BPC, h: int = H, w: int = W, reps: int = 1) -> bass.Bass:
    f32 = mybir.dt.float32
    f16 = mybir.dt.float16
    f8 = mybir.dt.float8e4
    DR = mybir.MatmulPerfMode.DoubleRow
    hp, wp = h + 2, w + 2
    assert h % R == 0
    nblocks = h // R

    nc = bacc.Bacc("TRN2", target_bir_lowering=False, debug=False)
    x_in = nc.declare_dram_parameter("x", [bpc, C, h, w], f16, isOutput=False)
    w_in = nc.declare_dram_parameter("w", [C, 9, 2, C], f8, isOutput=False)
    out_d = nc.declare_dram_parameter("out", [bpc, C, h, w], f16, isOutput=True)

    with tile.TileContext(nc) as tc, ExitStack() as ctx:
        wpool = ctx.enter_context(tc.tile_pool(name="wpool", bufs=1))
        apool = ctx.enter_context(tc.tile_pool(name="apool", bufs=2))
        opool = ctx.enter_context(tc.tile_pool(name="opool", bufs=6))
        pspool = ctx.enter_context(tc.tile_pool(name="pspool", bufs=1, space="PSUM"))

        w_s = wpool.tile([C, 9, 2, C], f8)
        nc.sync.dma_start(w_s[:], w_in[:])

        # PE warmup: the HAM clock gate holds the PE at low clock until it has
        # been busy ~3.4 us.  A few dummy matmuls during the input-DMA lead-in
        # (PE would be idle anyway) un-throttle it before the real work lands.
        warm = wpool.tile([C, R * w], f8)
        nc.vector.memset(warm[:], 0.0)
        wps = pspool.tile([C, R * w], f32, tag="ps0", name="wps")
        for _wi in range(6):
            nc.tensor.matmul(wps[:], warm[:, :C], warm[:], start=True, stop=True)

        for b_outer in range(bpc * reps):
            b = b_outer % bpc
            a_raw = apool.tile([C, h, w], f16, tag="a_raw", bufs=2)
            a = apool.tile([C, 2, hp, wp], f8, tag="a_pad", bufs=4)
            # zero the 1-wide border of both planes
            nc.vector.memset(a[:, :, 0, :], 0.0)
            nc.vector.memset(a[:, :, hp - 1, :], 0.0)
            nc.vector.memset(a[:, :, 1 : hp - 1, 0], 0.0)
            nc.vector.memset(a[:, :, 1 : hp - 1, wp - 1], 0.0)
            # image 0 leads with smaller chunks so the first matmul group can
            # start earlier; later images are fully overlapped with compute
            if h >= 32 and b_outer == 0:
                sched = [2, 2, 4, 8] + [16] * ((h - 16) // 16)
            elif h >= 32:
                sched = [16] * (h // 16)
            else:
                sched = [h]
            bounds = [0]
            for sz in sched:
                bounds.append(bounds[-1] + sz)
            for ci in range(len(sched)):
                r0, r1 = bounds[ci], bounds[ci + 1]
                nc.sync.dma_start(a_raw[:, r0:r1, :], x_in[b][:, r0:r1, :])
                # plane0 = e4m3(clip(x)) on DVE
                nc.vector.tensor_scalar(
                    out=a[:, 0, 1 + r0 : 1 + r1, 1 : wp - 1], in0=a_raw[:, r0:r1, :],
                    scalar1=1.0, scalar2=-1.0,
                    op0=mybir.AluOpType.min, op1=mybir.AluOpType.max,
                )
                # plane1 = plane0 / 16, also on DVE (same engine as the clip
                # => program order, no cross-engine sync; ACT keeps the
                # PSUM-drain lane to itself)
                nc.vector.tensor_scalar(
                    out=a[:, 1, 1 + r0 : 1 + r1, 1 : wp - 1],
                    in0=a[:, 0, 1 + r0 : 1 + r1, 1 : wp - 1],
                    scalar1=0.0625, scalar2=None,
                    op0=mybir.AluOpType.mult,
                )

            out_flat = out_d[b].rearrange("c h w -> c (h w)")
            # group schedule: image 0 ramps up with small groups
            if b_outer == 0:
                gsched = [1, 1, 2, 4] + [G] * ((nblocks - 8) // G)
            else:
                gsched = [G] * (nblocks // G)
            while sum(gsched) < nblocks:
                gsched.append(nblocks - sum(gsched))
            g0 = 0
            for gsz in gsched:
                blks = list(range(g0, g0 + gsz))
                g0 += gsz
                pss = []
                for j, blk in enumerate(blks):
                    ps_t = pspool.tile([C, R * w], f32, tag=f"ps{j}", name=f"ps{j}_")
                    pss.append(ps_t)
                for i, (kh, kw) in enumerate(TAPS):
                    lhsT = w_s[:, (kh + 1) * 3 + (kw + 1), :, :]
                    for j, blk in enumerate(blks):
                        h0 = blk * R
                        # trim rows that would only read the zero pad (image
                        # top/bottom edges); the trimmed PSUM slice stays flat
                        # because only whole rows are dropped
                        rr0 = max(0, -(h0 + kh))
                        rr1 = min(R, h - h0 - kh)
                        rhs = a[:, :, h0 + kh + 1 + rr0 : h0 + kh + 1 + rr1,
                                kw + 1 : kw + 1 + w]
                        inst = nc.tensor.matmul(
                            pss[j][:, rr0 * w : rr1 * w], lhsT, rhs,
                            start=(i == 0), stop=(i == len(TAPS) - 1),
                            perf_mode=DR,
                        )
                        if j > 0:
                            inst.ldweights = False
                for j, blk in enumerate(blks):
                    h0 = blk * R
                    ot = opool.tile([C, R * w], f16, tag="ot")
                    nc.scalar.copy(ot[:], pss[j][:])
                    nc.scalar.dma_start(out_flat[:, h0 * w : (h0 + R) * w], ot[:])

    nc.compile()
    return nc


def _build_fp16(bpc: int = BPC, h: int = H, w: int = W, reps: int = 1) -> bass.Bass:
    f32 = mybir.dt.float32
    wdt = mybir.dt.float16
    hp, wp = h + 2, w + 2
    assert h % R == 0

    nc = bacc.Bacc("TRN2", target_bir_lowering=False, debug=False)
    x_in = nc.declare_dram_parameter("x", [bpc, C, h, w], f32, isOutput=False)
    w_in = nc.declare_dram_parameter("w", [C, 9, C], wdt, isOutput=False)
    out_d = nc.declare_dram_parameter("out", [bpc, C, h, w], f32, isOutput=True)

    with tile.TileContext(nc) as tc, ExitStack() as ctx:
        wpool = ctx.enter_context(tc.tile_pool(name="wpool", bufs=1))
        apool = ctx.enter_context(tc.tile_pool(name="apool", bufs=2))
        opool = ctx.enter_context(tc.tile_pool(name="opool", bufs=6))
        pspool = ctx.enter_context(tc.tile_pool(name="pspool", bufs=8, space="PSUM"))

        w_s = wpool.tile([C, 9, C], wdt)
        nc.sync.dma_start(w_s[:], w_in[:])

        warm = wpool.tile([C, R * w], wdt)
        nc.vector.memset(warm[:], 0.0)
        wps = pspool.tile([C, R * w], f32, tag="ps")
        for _wi in range(6):
            nc.tensor.matmul(wps[:], warm[:, :C], warm[:], start=True, stop=True)

        for b_outer in range(bpc * reps):
            b = b_outer % bpc
            a_raw = apool.tile([C, h, w], f32, tag="a_raw", bufs=1)
            a = apool.tile([C, hp, wp], wdt, tag="a_pad", bufs=4)
            nc.vector.memset(a[:, 0, :], 0.0)
            nc.vector.memset(a[:, hp - 1, :], 0.0)
            nc.vector.memset(a[:, 1 : hp - 1, 0], 0.0)
            nc.vector.memset(a[:, 1 : hp - 1, wp - 1], 0.0)
            if h >= 32 and b_outer == 0:
                sched = [2, 2, 4, 8] + [16] * ((h - 16) // 16)
            elif h >= 32:
                sched = [16] * (h // 16)
            else:
                sched = [h]
            bounds = [0]
            for sz in sched:
                bounds.append(bounds[-1] + sz)
            for ci in range(len(sched)):
                r0, r1 = bounds[ci], bounds[ci + 1]
                nc.sync.dma_start(a_raw[:, r0:r1, :], x_in[b][:, r0:r1, :])
                nc.vector.tensor_scalar(
                    out=a[:, 1 + r0 : 1 + r1, 1 : wp - 1], in0=a_raw[:, r0:r1, :],
                    scalar1=1.0, scalar2=-1.0,
                    op0=mybir.AluOpType.min, op1=mybir.AluOpType.max,
                )

            out_flat = out_d[b].rearrange("c h w -> c (h w)")
            for h0 in range(0, h, R):
                ps = pspool.tile([C, R * w], f32)
                for i, (kh, kw) in enumerate(TAPS):
                    rr0 = max(0, -(h0 + kh))
                    rr1 = min(R, h - h0 - kh)
                    rhs = a[:, h0 + kh + 1 + rr0 : h0 + kh + 1 + rr1,
                            kw + 1 : kw + 1 + w]
                    lhsT = w_s[:, (kh + 1) * 3 + (kw + 1), :]
                    nc.tensor.matmul(
                        ps[:, rr0 * w : rr1 * w], lhsT, rhs,
                        start=(i == 0), stop=(i == len(TAPS) - 1),
                    )

                ot = opool.tile([C, R * w], f32)
                nc.scalar.copy(ot[:], ps[:])
                nc.scalar.dma_start(out_flat[:, h0 * w : (h0 + R) * w], ot[:])

    nc.compile()
    return nc


def _build(mode: str, bpc: int = BPC, h: int = H, w: int = W, reps: int = 1) -> bass.Bass:
    if mode == "fp8dr":
        return _build_fp8dr(bpc=bpc, h=h, w=w, reps=reps)
    return _build_fp16(bpc=bpc, h=h, w=w, reps=reps)


def _prep_weights(reweight: np.ndarray, conv_w: np.ndarray, mode: str) -> np.ndarray:
    w_full = np.concatenate([np.clip(reweight, -1.0, 1.0), conv_w], axis=0)  # [128,128,3,3]
    w_t = np.ascontiguousarray(w_full.transpose(1, 2, 3, 0)).reshape(C, 9, C)  # [ci,tap,co]
    if mode == "fp8dr":
        w_t = w_t.astype(np.float32)
        whi = w_t.astype(F8)
        wlo = ((w_t - whi.astype(np.float32)) * 16.0).astype(F8)
        return np.ascontiguousarray(np.stack([whi, wlo], axis=2))  # [ci,tap,2,co]
    return w_t.astype(np.float16)


def _make_in_maps(x: np.ndarray, reweight: np.ndarray, conv_w: np.ndarray, mode: str):
    w_k = _prep_weights(np.asarray(reweight), np.asarray(conv_w), mode)
    xdt = np.float16 if mode == "fp8dr" else np.float32
    x = np.asarray(x).astype(xdt)
    return [
        {"x": np.ascontiguousarray(x[i * BPC : (i + 1) * BPC]), "w": w_k}
        for i in range(NCORES)
    ]


def _get_nc(mode: str):
    if mode not in _nc_cache:
        _nc_cache[mode] = _build(mode)
    return _nc_cache[mode]


class _Runner:
    """Persistent jitted shard_map executor for one compiled Bass module.

    Mirrors concourse.bass2jax.run_bass_via_pjrt's multi-core path, but keeps
    the jitted function (and the donated output buffers) alive across calls so
    repeated kernel() invocations skip recompilation.  Output buffers are
    donation-chained: the kernel writes every output element, so reusing the
    previous call's outputs as the next call's output buffers is safe.
    """

    def __init__(self, nc, n_cores: int):
        import jax
        from concourse import bass2jax
        from jax.experimental.shard_map import shard_map
        from jax.sharding import Mesh, NamedSharding, PartitionSpec

        bass2jax.install_neuronx_cc_hook()
        self.jax = jax
        self.n_cores = n_cores
        partition_name = nc.partition_id_tensor.name if nc.partition_id_tensor else None
        in_names, out_names, out_avals = [], [], []
        for alloc in nc.m.functions[0].allocations:
            if not isinstance(alloc, mybir.MemoryLocationSet):
                continue
            name = alloc.memorylocations[0].name
            if alloc.kind == "ExternalInput":
                if name != partition_name:
                    in_names.append(name)
            elif alloc.kind == "ExternalOutput":
                out_names.append(name)
                out_avals.append(
                    jax.core.ShapedArray(
                        tuple(alloc.tensor_shape), mybir.dt.np(alloc.dtype)
                    )
                )
        self.in_names, self.out_names, self.out_avals = in_names, out_names, out_avals
        n_params = len(in_names)
        all_in_names = list(in_names) + list(out_names)
        if partition_name is not None:
            all_in_names.append(partition_name)
        donate = tuple(range(n_params, n_params + len(out_names)))

        def _body(*args):
            operands = list(args)
            if partition_name is not None:
                operands.append(bass2jax.partition_id_tensor())
            return tuple(
                bass2jax._bass_exec_p.bind(
                    *operands,
                    out_avals=tuple(out_avals),
                    in_names=tuple(all_in_names),
                    out_names=tuple(out_names),
                    lowering_input_output_aliases=(),
                    sim_require_finite=True,
                    sim_require_nnan=True,
                    nc=nc,
                )
            )

        devices = jax.devices()[:n_cores]
        assert len(devices) >= n_cores, f"need {n_cores} devices, got {len(devices)}"
        mesh = Mesh(np.asarray(devices), ("core",))
        self.sharding = NamedSharding(mesh, PartitionSpec("core"))
        self.sharded = jax.jit(
            shard_map(
                _body, mesh=mesh,
                in_specs=(PartitionSpec("core"),) * (n_params + len(out_names)),
                out_specs=(PartitionSpec("core"),) * len(out_names),
                check_rep=False,
            ),
            donate_argnums=donate, keep_unused=True,
        )
        self._outs = None  # donated output buffers, chained across calls

    def __call__(self, in_maps):
        jax = self.jax
        per_core = [[np.asarray(m[name]) for name in self.in_names] for m in in_maps]
        concat_in = [
            np.concatenate([per_core[c][i] for c in range(self.n_cores)], axis=0)
            for i in range(len(self.in_names))
        ]
        xin = [jax.device_put(a, self.sharding) for a in concat_in]
        if self._outs is None:
            self._outs = [
                jax.device_put(
                    np.zeros((self.n_cores * av.shape[0], *av.shape[1:]), av.dtype),
                    self.sharding,
                )
                for av in self.out_avals
            ]
        self._outs = list(self.sharded(*xin, *self._outs))
        out_np = [np.asarray(o) for o in self._outs]
        return [
            {
                name: out_np[i].reshape(self.n_cores, *self.out_avals[i].shape)[c]
                for i, name in enumerate(self.out_names)
            }
            for c in range(self.n_cores)
        ]


def _run_spmd(nc, in_maps, mode: str):
    last = None
    for attempt in range(3):
        try:
            if mode not in _runner_cache:
                _runner_cache[mode] = _Runner(nc, NCORES)
            return _runner_cache[mode](in_maps)
        except Exception as e:
            last = e
            _runner_cache.pop(mode, None)
        # fall back to the stock one-shot path (also covers transient
        # device/terminal wedges, with a pause between attempts)
        try:
            from concourse.bass_utils import run_bass_kernel_spmd

            return run_bass_kernel_spmd(nc, in_maps, list(range(NCORES))).results
        except Exception as e:
            last = e
            _time.sleep(15)
    raise last


def run(x, reweight, conv_w, mode: str | None = None):
    mode = mode or MODE
    nc = _get_nc(mode)
    in_maps = _make_in_maps(x, reweight, conv_w, mode)
    results = _run_spmd(nc, in_maps, mode)
    out = np.concatenate([results[i]["out"] for i in range(NCORES)], axis=0)
    if out.dtype != np.float32:
        out = out.astype(np.float32)
    return out


def kernel(x, reweight, conv_w):
    return run(x, reweight, conv_w)


# revision 8
# speedup vs baseline: 1.2135x; 1.0819x over previous
"""Binarized 3x3 conv2d (hardtanh activation, clipped reweight ++ plain conv_w)
run data-parallel across 8 Trainium2 NeuronCores.

Math: out = conv2d(clip(x,-1,1), concat(clip(reweight,-1,1), conv_w)), pad=1
— a single 128->128 channel 3x3 conv (the two branches just split the output
channels), so the weights are fused + pre-transposed on the host and the conv
runs as one kernel.

Sharding: pure data parallel — batch 32 -> 4 images per core, weights
replicated (tiny). No collectives needed (forward only).

Mode "fp8dr" (default): self-correcting fp8 DoubleRow conv.
  The PE's only >1x matmul mode on TRN2 is fp8e4/e5 DoubleRow (2 MACs/cell/
  cycle, K virtually 256).  Plain e4m3 quantization of weights+activations
  fails the 2e-2 absmax-rel budget (measured 3.0e-2, dominated by the weight
  quantization).  Instead each DoubleRow matmul computes BOTH the fp8 product
  and its first-order weight-error correction in one pass:
      slot0: Whi^T  @ a8        with Whi = e4m3(W)
      slot1: Wlo^T  @ (a8/16)   with Wlo = e4m3(16*(W - Whi))
  = (Whi + (W-Whi))^T a8 exactly (to second order), so only the activation
  quantization error remains (measured 1.4e-2 < 2e-2).  Activations live as a
  two-plane fp8 image [C, 2, 114, 114]: plane0 = e4m3(clip(x)) (DVE clip),
  plane1 = plane0/16 (ACT scale-copy).  Each of the 9 taps is one DoubleRow
  matmul whose moving operand is the 4D AP [C, 2(plane), rows, 112].

  LDWEIGHTS in DoubleRow mode can't use Fast Weight Load (256 cols, ~256 cyc)
  and would eat the 2x gain, so blocks are processed in groups of 8 (one PSUM
  bank each) with the tap loop outer: the first matmul of a tap self-loads,
  the other 7 set InstMatmult.ldweights=False and reuse the loaded weights
  (measured: 148.6us fp16 -> 103.8us for the grouped DR inner loop).

  I/O is fp16 (pure dtype cast on host, float16 in / float16 out) to keep DMA
  at ~26MB/core, under the PE time.  PSUM (fp32) is drained by ACT copies to
  fp16 SBUF tiles and DMA'd out from there (same engine => program-order WAR,
  keeps every instruction within the 2-sync-command hardware limit).

Mode "fp16": previous baseline (9 normal fp16 matmuls per block, fp32 I/O),
  kept as fallback.  Measured 163.6us this session.
"""

import time as _time

import numpy as np
import ml_dtypes
from contextlib import ExitStack

import concourse.bass as bass
import concourse.mybir as mybir
import concourse.tile as tile
from concourse import bacc

B, C, H, W = 32, 128, 112, 112
NCORES = 8
BPC = B // NCORES  # images per core
R = 4              # output rows per PSUM block (R*W = 448 <= 512 psum bank)
G = 8              # blocks per ldweights-amortization group (= PSUM banks)

MODE = "fp8dr"  # "fp8dr" | "fp16"

F8 = ml_dtypes.float8_e4m3  # TRN FP8_EXP4-compatible for |v| <= 240

_nc_cache: dict = {}
_runner_cache: dict = {}

# (0,0) first: it is never row-trimmed, so the start=True matmul writes
# every element of the PSUM block before partial taps accumulate into it
TAPS = [(0, 0)] + [
    (kh, kw) for kh in (-1, 0, 1) for kw in (-1, 0, 1) if (kh, kw) != (0, 0)
]


def _build_fp8dr(bpc: int = BPC, h: int = H, w: int = W, reps: int = 1) -> bass.Bass:
    f32 = mybir.dt.float32
    f16 = mybir.dt.float16
    f8 = mybir.dt.float8e4
    DR = mybir.MatmulPerfMode.DoubleRow
    hp, wp = h + 2, w + 2
    assert h % R == 0
    nblocks = h // R

    nc = bacc.Bacc("TRN2", target_bir_lowering=False, debug=False)
    # x arrives zero-PADDED from the host ([hp, wp] spatial): the clip pass
    # then covers pad and interior uniformly (clip(0)=0), every DVE op is a
    # dense unit-stride full-row op, and no border memsets are needed.
    x_in = nc.declare_dram_parameter("x", [bpc, C, hp, wp], f16, isOutput=False)
    w_in = nc.declare_dram_parameter("w", [C, 9, 2, C], f8, isOutput=False)
    out_d = nc.declare_dram_parameter("out", [bpc, C, h, w], f16, isOutput=True)

    with tile.TileContext(nc) as tc, ExitStack() as ctx:
        wpool = ctx.enter_context(tc.tile_pool(name="wpool", bufs=1))
        apool = ctx.enter_context(tc.tile_pool(name="apool", bufs=2))
        opool = ctx.enter_context(tc.tile_pool(name="opool", bufs=6))
        pspool = ctx.enter_context(tc.tile_pool(name="pspool", bufs=1, space="PSUM"))

        w_s = wpool.tile([C, 9, 2, C], f8)
        nc.sync.dma_start(w_s[:], w_in[:])

        # PE warmup: the HAM clock gate holds the PE at low clock until it has
        # been busy ~3.4 us.  A few dummy matmuls during the input-DMA lead-in
        # (PE would be idle anyway) un-throttle it before the real work lands.
        warm = wpool.tile([C, R * w], f8)
        nc.vector.memset(warm[:], 0.0)
        wps = pspool.tile([C, R * w], f32, tag="ps0", name="wps")
        for _wi in range(6):
            nc.tensor.matmul(wps[:], warm[:, :C], warm[:], start=True, stop=True)

        for b_outer in range(bpc * reps):
            b = b_outer % bpc
            a_raw = apool.tile([C, hp, wp], f16, tag="a_raw", bufs=2)
            a = apool.tile([C, 2, hp, wp], f8, tag="a_pad", bufs=4)
            # image 0 leads with smaller chunks so the first matmul group can
            # start earlier; later images are fully overlapped with compute
            # (chunks are in padded-row coordinates, covering all hp rows)
            if b_outer == 0:
                sched = [4, 4, 8] + [16] * ((hp - 16) // 16)
            else:
                sched = [16] * (hp // 16)
            while sum(sched) < hp:
                sched.append(hp - sum(sched))
            bounds = [0]
            for sz in sched:
                bounds.append(bounds[-1] + sz)
            for ci in range(len(sched)):
                r0, r1 = bounds[ci], bounds[ci + 1]
                nc.sync.dma_start(a_raw[:, r0:r1, :], x_in[b][:, r0:r1, :])
                # plane0 = e4m3(clip(x)) on DVE — dense unit-stride rows
                nc.vector.tensor_scalar(
                    out=a[:, 0, r0:r1, :], in0=a_raw[:, r0:r1, :],
                    scalar1=1.0, scalar2=-1.0,
                    op0=mybir.AluOpType.min, op1=mybir.AluOpType.max,
                )
                # plane1 = plane0 / 16, also on DVE (same engine as the clip
                # => program order, no cross-engine sync; ACT keeps the
                # PSUM-drain lane to itself)
                nc.vector.tensor_scalar(
                    out=a[:, 1, r0:r1, :], in0=a[:, 0, r0:r1, :],
                    scalar1=0.0625, scalar2=None,
                    op0=mybir.AluOpType.mult,
                )

            out_flat = out_d[b].rearrange("c h w -> c (h w)")
            # group schedule: image 0 ramps up with small groups
            if b_outer == 0:
                gsched = [1, 1, 2, 4] + [G] * ((nblocks - 8) // G)
            else:
                gsched = [G] * (nblocks // G)
            while sum(gsched) < nblocks:
                gsched.append(nblocks - sum(gsched))
            g0 = 0
            for gsz in gsched:
                blks = list(range(g0, g0 + gsz))
                g0 += gsz
                pss = []
                for j, blk in enumerate(blks):
                    ps_t = pspool.tile([C, R * w], f32, tag=f"ps{j}", name=f"ps{j}_")
                    pss.append(ps_t)
                for i, (kh, kw) in enumerate(TAPS):
                    lhsT = w_s[:, (kh + 1) * 3 + (kw + 1), :, :]
                    for j, blk in enumerate(blks):
                        h0 = blk * R
                        # trim rows that would only read the zero pad (image
                        # top/bottom edges); the trimmed PSUM slice stays flat
                        # because only whole rows are dropped
                        rr0 = max(0, -(h0 + kh))
                        rr1 = min(R, h - h0 - kh)
                        rhs = a[:, :, h0 + kh + 1 + rr0 : h0 + kh + 1 + rr1,
                                kw + 1 : kw + 1 + w]
                        inst = nc.tensor.matmul(
                            pss[j][:, rr0 * w : rr1 * w], lhsT, rhs,
                            start=(i == 0), stop=(i == len(TAPS) - 1),
                            perf_mode=DR,
                        )
                        if j > 0:
                            inst.ldweights = False
                # drain the whole group into one contiguous fp16 staging tile,
                # then ONE dma_start for the group: per-block DMA triggers
                # (448/kernel) would monopolize the ACT sequencer's
                # descriptor generation and starve the drains behind them
                ot = opool.tile([C, G * R * w], f16, tag="ot", bufs=3)
                for j, blk in enumerate(blks):
                    nc.scalar.copy(ot[:, j * R * w : (j + 1) * R * w], pss[j][:])
                g_lo = blks[0] * R * w
                g_hi = (blks[-1] + 1) * R * w
                nc.scalar.dma_start(out_flat[:, g_lo:g_hi], ot[:, : g_hi - g_lo])

    nc.compile()
    return nc


def _build_fp16(bpc: int = BPC, h: int = H, w: int = W, reps: int = 1) -> bass.Bass:
    f32 = mybir.dt.float32
    wdt = mybir.dt.float16
    hp, wp = h + 2, w + 2
    assert h % R == 0

    nc = bacc.Bacc("TRN2", target_bir_lowering=False, debug=False)
    x_in = nc.declare_dram_parameter("x", [bpc, C, h, w], f32, isOutput=False)
    w_in = nc.declare_dram_parameter("w", [C, 9, C], wdt, isOutput=False)
    out_d = nc.declare_dram_parameter("out", [bpc, C, h, w], f32, isOutput=True)

    with tile.TileContext(nc) as tc, ExitStack() as ctx:
        wpool = ctx.enter_context(tc.tile_pool(name="wpool", bufs=1))
        apool = ctx.enter_context(tc.tile_pool(name="apool", bufs=2))
        opool = ctx.enter_context(tc.tile_pool(name="opool", bufs=6))
        pspool = ctx.enter_context(tc.tile_pool(name="pspool", bufs=8, space="PSUM"))

        w_s = wpool.tile([C, 9, C], wdt)
        nc.sync.dma_start(w_s[:], w_in[:])

        warm = wpool.tile([C, R * w], wdt)
        nc.vector.memset(warm[:], 0.0)
        wps = pspool.tile([C, R * w], f32, tag="ps")
        for _wi in range(6):
            nc.tensor.matmul(wps[:], warm[:, :C], warm[:], start=True, stop=True)

        for b_outer in range(bpc * reps):
            b = b_outer % bpc
            a_raw = apool.tile([C, h, w], f32, tag="a_raw", bufs=1)
            a = apool.tile([C, hp, wp], wdt, tag="a_pad", bufs=4)
            nc.vector.memset(a[:, 0, :], 0.0)
            nc.vector.memset(a[:, hp - 1, :], 0.0)
            nc.vector.memset(a[:, 1 : hp - 1, 0], 0.0)
            nc.vector.memset(a[:, 1 : hp - 1, wp - 1], 0.0)
            if h >= 32 and b_outer == 0:
                sched = [2, 2, 4, 8] + [16] * ((h - 16) // 16)
            elif h >= 32:
                sched = [16] * (h // 16)
            else:
                sched = [h]
            bounds = [0]
            for sz in sched:
                bounds.append(bounds[-1] + sz)
            for ci in range(len(sched)):
                r0, r1 = bounds[ci], bounds[ci + 1]
                nc.sync.dma_start(a_raw[:, r0:r1, :], x_in[b][:, r0:r1, :])
                nc.vector.tensor_scalar(
                    out=a[:, 1 + r0 : 1 + r1, 1 : wp - 1], in0=a_raw[:, r0:r1, :],
                    scalar1=1.0, scalar2=-1.0,
                    op0=mybir.AluOpType.min, op1=mybir.AluOpType.max,
                )

            out_flat = out_d[b].rearrange("c h w -> c (h w)")
            for h0 in range(0, h, R):
                ps = pspool.tile([C, R * w], f32)
                for i, (kh, kw) in enumerate(TAPS):
                    rr0 = max(0, -(h0 + kh))
                    rr1 = min(R, h - h0 - kh)
                    rhs = a[:, h0 + kh + 1 + rr0 : h0 + kh + 1 + rr1,
                            kw + 1 : kw + 1 + w]
                    lhsT = w_s[:, (kh + 1) * 3 + (kw + 1), :]
                    nc.tensor.matmul(
                        ps[:, rr0 * w : rr1 * w], lhsT, rhs,
                        start=(i == 0), stop=(i == len(TAPS) - 1),
                    )

                ot = opool.tile([C, R * w], f32)
                nc.scalar.copy(ot[:], ps[:])
                nc.scalar.dma_start(out_flat[:, h0 * w : (h0 + R) * w], ot[:])

    nc.compile()
    return nc


def _build(mode: str, bpc: int = BPC, h: int = H, w: int = W, reps: int = 1) -> bass.Bass:
    if mode == "fp8dr":
        return _build_fp8dr(bpc=bpc, h=h, w=w, reps=reps)
    return _build_fp16(bpc=bpc, h=h, w=w, reps=reps)


def _prep_weights(reweight: np.ndarray, conv_w: np.ndarray, mode: str) -> np.ndarray:
    w_full = np.concatenate([np.clip(reweight, -1.0, 1.0), conv_w], axis=0)  # [128,128,3,3]
    w_t = np.ascontiguousarray(w_full.transpose(1, 2, 3, 0)).reshape(C, 9, C)  # [ci,tap,co]
    if mode == "fp8dr":
        w_t = w_t.astype(np.float32)
        whi = w_t.astype(F8)
        wlo = ((w_t - whi.astype(np.float32)) * 16.0).astype(F8)
        return np.ascontiguousarray(np.stack([whi, wlo], axis=2))  # [ci,tap,2,co]
    return w_t.astype(np.float16)


def _make_in_maps(x: np.ndarray, reweight: np.ndarray, conv_w: np.ndarray, mode: str):
    w_k = _prep_weights(np.asarray(reweight), np.asarray(conv_w), mode)
    if mode == "fp8dr":
        xp = np.zeros((B, C, H + 2, W + 2), dtype=np.float16)
        xp[:, :, 1 : H + 1, 1 : W + 1] = np.asarray(x).astype(np.float16)
        x = xp
    else:
        x = np.asarray(x).astype(np.float32)
    return [
        {"x": np.ascontiguousarray(x[i * BPC : (i + 1) * BPC]), "w": w_k}
        for i in range(NCORES)
    ]


def _get_nc(mode: str):
    if mode not in _nc_cache:
        _nc_cache[mode] = _build(mode)
    return _nc_cache[mode]


class _Runner:
    """Persistent jitted shard_map executor for one compiled Bass module.

    Mirrors concourse.bass2jax.run_bass_via_pjrt's multi-core path, but keeps
    the jitted function (and the donated output buffers) alive across calls so
    repeated kernel() invocations skip recompilation.  Output buffers are
    donation-chained: the kernel writes every output element, so reusing the
    previous call's outputs as the next call's output buffers is safe.
    """

    def __init__(self, nc, n_cores: int):
        import jax
        from concourse import bass2jax
        from jax.experimental.shard_map import shard_map
        from jax.sharding import Mesh, NamedSharding, PartitionSpec

        bass2jax.install_neuronx_cc_hook()
        self.jax = jax
        self.n_cores = n_cores
        partition_name = nc.partition_id_tensor.name if nc.partition_id_tensor else None
        in_names, out_names, out_avals = [], [], []
        for alloc in nc.m.functions[0].allocations:
            if not isinstance(alloc, mybir.MemoryLocationSet):
                continue
            name = alloc.memorylocations[0].name
            if alloc.kind == "ExternalInput":
                if name != partition_name:
                    in_names.append(name)
            elif alloc.kind == "ExternalOutput":
                out_names.append(name)
                out_avals.append(
                    jax.core.ShapedArray(
                        tuple(alloc.tensor_shape), mybir.dt.np(alloc.dtype)
                    )
                )
        self.in_names, self.out_names, self.out_avals = in_names, out_names, out_avals
        n_params = len(in_names)
        all_in_names = list(in_names) + list(out_names)
        if partition_name is not None:
            all_in_names.append(partition_name)
        donate = tuple(range(n_params, n_params + len(out_names)))

        def _body(*args):
            operands = list(args)
            if partition_name is not None:
                operands.append(bass2jax.partition_id_tensor())
            return tuple(
                bass2jax._bass_exec_p.bind(
                    *operands,
                    out_avals=tuple(out_avals),
                    in_names=tuple(all_in_names),
                    out_names=tuple(out_names),
                    lowering_input_output_aliases=(),
                    sim_require_finite=True,
                    sim_require_nnan=True,
                    nc=nc,
                )
            )

        devices = jax.devices()[:n_cores]
        assert len(devices) >= n_cores, f"need {n_cores} devices, got {len(devices)}"
        mesh = Mesh(np.asarray(devices), ("core",))
        self.sharding = NamedSharding(mesh, PartitionSpec("core"))
        self.sharded = jax.jit(
            shard_map(
                _body, mesh=mesh,
                in_specs=(PartitionSpec("core"),) * (n_params + len(out_names)),
                out_specs=(PartitionSpec("core"),) * len(out_names),
                check_rep=False,
            ),
            donate_argnums=donate, keep_unused=True,
        )
        self._outs = None  # donated output buffers, chained across calls

    def __call__(self, in_maps):
        jax = self.jax
        per_core = [[np.asarray(m[name]) for name in self.in_names] for m in in_maps]
        concat_in = [
            np.concatenate([per_core[c][i] for c in range(self.n_cores)], axis=0)
            for i in range(len(self.in_names))
        ]
        xin = [jax.device_put(a, self.sharding) for a in concat_in]
        if self._outs is None:
            self._outs = [
                jax.device_put(
                    np.zeros((self.n_cores * av.shape[0], *av.shape[1:]), av.dtype),
                    self.sharding,
                )
                for av in self.out_avals
            ]
        self._outs = list(self.sharded(*xin, *self._outs))
        out_np = [np.asarray(o) for o in self._outs]
        return [
            {
                name: out_np[i].reshape(self.n_cores, *self.out_avals[i].shape)[c]
                for i, name in enumerate(self.out_names)
            }
            for c in range(self.n_cores)
        ]


def _run_spmd(nc, in_maps, mode: str):
    last = None
    for attempt in range(3):
        try:
            if mode not in _runner_cache:
                _runner_cache[mode] = _Runner(nc, NCORES)
            return _runner_cache[mode](in_maps)
        except Exception as e:
            last = e
            _runner_cache.pop(mode, None)
        # fall back to the stock one-shot path (also covers transient
        # device/terminal wedges, with a pause between attempts)
        try:
            from concourse.bass_utils import run_bass_kernel_spmd

            return run_bass_kernel_spmd(nc, in_maps, list(range(NCORES))).results
        except Exception as e:
            last = e
            _time.sleep(15)
    raise last


def run(x, reweight, conv_w, mode: str | None = None):
    mode = mode or MODE
    nc = _get_nc(mode)
    in_maps = _make_in_maps(x, reweight, conv_w, mode)
    results = _run_spmd(nc, in_maps, mode)
    out = np.concatenate([results[i]["out"] for i in range(NCORES)], axis=0)
    if out.dtype != np.float32:
        out = out.astype(np.float32)
    return out


def kernel(x, reweight, conv_w):
    return run(x, reweight, conv_w)
